# revision 1
# baseline (speedup 1.0000x reference)
"""Trainium2 Bass kernel for the GNN edge-update MLP (8 NeuronCores).

Reference semantics:
    h   = x @ W_lin.T + b_lin                       # [N, nin]
    agg = h[src] + h[dst]                           # [E, nin]
    z   = concat([agg, edge_attr], -1)              # [E, 2*nin]
    z   = relu(BN(z @ W1.T + b1; g1, be1))          # [E, nout]  (BN over edges)
    z   = relu(BN(z @ W2.T + b2; g2, be2))          # [E, nout]

Restructuring:
  * b1/b2 cancel inside training-mode BN -> dropped.
  * z @ W1.T = hW[src] + hW[dst] + ea @ W1b.T, with W1 = [W1a | W1b] and
    hW = x @ (W1a W_lin).T + W1a b_lin  (a [N, nout] gather table).
  * Everything on device is feature-major [128, edges]; host pre-transposes
    edge_attr / x and post-transposes the output.
  * Gathers use GPSIMD dma_gather(transpose=True): int16 indices (signed on
    HW), so the table is built in two regions (hi nodes first, then lo) and
    the host bucket-sorts each core's edges by (src>=SPLIT, dst>=SPLIT) so
    every gather instruction targets one region with small non-negative
    local indices.  Buckets are padded (to the max size over cores) with
    edges that gather dedicated zero rows and have zero edge_attr, so padded
    u1 columns are exactly 0; their (constant) effect on the second BN's
    statistics is subtracted analytically on device.
  * BN statistics: per-chunk vector bn_stats, merged manually, AllReduce'd
    across the 8 cores ([128,2] f32 - tiny).
"""

import sys
from contextlib import ExitStack

import numpy as np

try:
    import concourse  # noqa: F401
except ImportError:  # pragma: no cover
    sys.path.insert(0, "/opt/trn_rl_repo")

import ml_dtypes
from concourse import bass, bacc, mybir
from concourse import tile
from concourse.bass_utils import run_bass_kernel_spmd
from concourse.masks import make_identity

BF16 = ml_dtypes.bfloat16

N_CORES = 8
NIN = 128
EPS = 1e-5
P = 128

SPLIT = 32767            # nodes < SPLIT are "lo", >= SPLIT are "hi"
BUCKET_ORDER = (3, 1, 2, 0)   # (hi,hi) first: table_hi builds fastest
GROUP = 2048             # edges per dma_gather instruction


def table_layout(n_nodes):
    """Two gather tables: hi = hW[SPLIT:] + zero row (padded to 512);
    lo = hW[0:SPLIT) + zero row.  xT columns: [hi | lo] in that order."""
    nhi = n_nodes - SPLIT
    hi_rows = ((nhi + 1 + 511) // 512) * 512
    lo_rows = ((SPLIT + 1 + 511) // 512) * 512
    npad = hi_rows + lo_rows
    return nhi, hi_rows, lo_rows, npad


def edge_layout(caps):
    """Device-side loop structure from bucket capacities.

    Returns (groups, chunks): groups = (off, L, src_hi, dst_hi);
    chunks = (off, S, group_index)."""
    groups = []
    chunks = []
    off = 0
    for b in BUCKET_ORDER:
        src_hi, dst_hi = b >= 2, b % 2 == 1
        rem = caps[b]
        while rem > 0:
            L = min(GROUP, rem)
            gi = len(groups)
            groups.append((off, L, src_hi, dst_hi))
            coff = 0
            while coff < L:
                S = min(512, L - coff)
                chunks.append((off + coff, S, gi))
                coff += S
            off += L
            rem -= L
    return groups, chunks


def _chunks(ec):
    out = []
    off = 0
    while off < ec:
        s = min(512, ec - off)
        out.append((off, s))
        off += s
    return out


def build_graph(n_cores, caps, n_nodes, e_total, eps=EPS):
    f32 = mybir.dt.float32
    bf16 = mybir.dt.bfloat16
    i16 = mybir.dt.int16
    FT = mybir.ActivationFunctionType

    nc = bacc.Bacc(
        "TRN2", target_bir_lowering=False, debug=False, num_devices=n_cores
    )

    nhi, hi_rows, lo_rows, npad = table_layout(n_nodes)
    groups, chunksA = edge_layout(caps)
    ec = sum(caps)
    chunksBC = _chunks(ec)
    nstat = max(len(chunksA), len(chunksBC))
    n_pad_tot = ec * n_cores - e_total  # padded edges across all cores

    # ---- I/O -------------------------------------------------------------
    eaT = nc.dram_tensor("eaT", [P, ec], bf16, kind="ExternalInput").ap()
    xT = nc.dram_tensor("xT", [P, npad], bf16, kind="ExternalInput").ap()
    sidx = nc.dram_tensor("sidx", [P, ec // 16], i16, kind="ExternalInput").ap()
    didx = nc.dram_tensor("didx", [P, ec // 16], i16, kind="ExternalInput").ap()
    wlin = nc.dram_tensor("wlin", [P, P], f32, kind="ExternalInput").ap()
    w1 = nc.dram_tensor("w1", [P, 2 * P], f32, kind="ExternalInput").ap()
    w2 = nc.dram_tensor("w2", [P, P], f32, kind="ExternalInput").ap()
    blin = nc.dram_tensor("blin", [P, 1], f32, kind="ExternalInput").ap()
    g1 = nc.dram_tensor("g1", [P, 1], f32, kind="ExternalInput").ap()
    be1 = nc.dram_tensor("be1", [P, 1], f32, kind="ExternalInput").ap()
    g2 = nc.dram_tensor("g2", [P, 1], f32, kind="ExternalInput").ap()
    be2 = nc.dram_tensor("be2", [P, 1], f32, kind="ExternalInput").ap()
    outT = nc.dram_tensor("outT", [P, ec], bf16, kind="ExternalOutput").ap()

    table_hi = nc.dram_tensor("hw_table_hi", [hi_rows, P], bf16).ap()
    table_lo = nc.dram_tensor("hw_table_lo", [lo_rows, P], bf16).ap()

    grp_all = [list(range(n_cores))]

    with tile.TileContext(nc) as tc, ExitStack() as es:
        consts = es.enter_context(tc.tile_pool(name="consts", bufs=1))
        gidx = es.enter_context(tc.tile_pool(name="gidx", bufs=4))
        dram = es.enter_context(tc.tile_pool(name="dram", bufs=1, space="DRAM"))
        big = es.enter_context(tc.tile_pool(name="big", bufs=1))
        red = es.enter_context(tc.tile_pool(name="red", bufs=1))

        # ---- constants / weight prep ------------------------------------
        ident_f = consts.tile([P, P], f32)
        make_identity(nc, ident_f[:])

        wlin_s = consts.tile([P, P], f32)
        nc.sync.dma_start(out=wlin_s[:], in_=wlin)
        w1_s = consts.tile([P, 2 * P], f32)
        nc.sync.dma_start(out=w1_s[:], in_=w1)
        w2_s = consts.tile([P, P], f32)
        nc.sync.dma_start(out=w2_s[:], in_=w2)
        blin_s = consts.tile([P, 1], f32)
        nc.sync.dma_start(out=blin_s[:], in_=blin)
        g1_s = consts.tile([P, 1], f32)
        nc.sync.dma_start(out=g1_s[:], in_=g1)
        be1_s = consts.tile([P, 1], f32)
        nc.sync.dma_start(out=be1_s[:], in_=be1)
        g2_s = consts.tile([P, 1], f32)
        nc.sync.dma_start(out=g2_s[:], in_=g2)
        be2_s = consts.tile([P, 1], f32)
        nc.sync.dma_start(out=be2_s[:], in_=be2)
        eps_s = consts.tile([P, 1], f32)
        nc.vector.memset(eps_s[:], eps)

        idx_pre = {}
        for gi, (off, L, _sh, _dh) in enumerate(groups[:4]):
            si = gidx.tile([P, GROUP // 16], i16, tag="si")
            nc.sync.dma_start(out=si[:, :L // 16],
                              in_=sidx[:, off // 16:(off + L) // 16])
            di = gidx.tile([P, GROUP // 16], i16, tag="di")
            nc.sync.dma_start(out=di[:, :L // 16],
                              in_=didx[:, off // 16:(off + L) // 16])
            idx_pre[gi] = (si, di)

        w1aT = consts.tile([P, P], f32)
        w1bT = consts.tile([P, P], bf16)
        w2T = consts.tile([P, P], bf16)
        wcT = consts.tile([P, P], bf16)
        bc = consts.tile([P, 1], f32)

        with tc.tile_pool(name="psum0", bufs=1, space="PSUM") as psw, \
             tc.tile_pool(name="psum0b", bufs=3, space="PSUM") as ps0:
            pw = psw.tile([P, P], f32, tag="pw")
            nc.tensor.matmul(pw[:], lhsT=w1_s[:, 0:P], rhs=ident_f[:],
                             start=True, stop=True)
            nc.vector.tensor_copy(w1aT[:], pw[:])
            pw = psw.tile([P, P], f32, tag="pw")
            nc.tensor.matmul(pw[:], lhsT=w1_s[:, P:2 * P], rhs=ident_f[:],
                             start=True, stop=True)
            nc.vector.tensor_copy(w1bT[:], pw[:])
            pw = psw.tile([P, P], f32, tag="pw")
            nc.tensor.matmul(pw[:], lhsT=w2_s[:], rhs=ident_f[:],
                             start=True, stop=True)
            nc.vector.tensor_copy(w2T[:], pw[:])
            # WcT[i, o] = (W1a @ W_lin)[o, i]
            pw = psw.tile([P, P], f32, tag="pw")
            nc.tensor.matmul(pw[:], lhsT=wlin_s[:], rhs=w1aT[:],
                             start=True, stop=True)
            nc.vector.tensor_copy(wcT[:], pw[:])
            pb = psw.tile([P, 1], f32, tag="pb")
            nc.tensor.matmul(pb[:], lhsT=w1aT[:], rhs=blin_s[:],
                             start=True, stop=True)
            nc.vector.tensor_copy(bc[:], pb[:])

            ident_b = consts.tile([P, P], bf16)
            nc.vector.tensor_copy(ident_b[:], ident_f[:])

            # ---- phase 0: build the gather tables (lo first) ------------
            with tc.tile_pool(name="ph0", bufs=4) as ph0:
                zrow = ph0.tile([P, P], bf16, tag="zrow")
                nc.vector.memset(zrow[:], 0.0)

                def build(tab, xcol0, nch):
                    for j in range(nch):
                        xt = ph0.tile([P, 512], bf16, tag="xt")
                        nc.sync.dma_start(
                            out=xt[:],
                            in_=xT[:, xcol0 + j * 512:xcol0 + (j + 1) * 512])
                        hp = ps0.tile([P, 512], f32, tag="hp")
                        nc.tensor.matmul(hp[:], lhsT=wcT[:], rhs=xt[:],
                                         start=True, stop=True)
                        hs = ph0.tile([P, 512], bf16, tag="hs")
                        nc.scalar.activation(hs[:], hp[:], func=FT.Identity,
                                             bias=bc[:], scale=1.0)
                        tp = ps0.tile([P, 512], f32, tag="tp")
                        for s in range(4):
                            nc.tensor.matmul(tp[:, s * P:(s + 1) * P],
                                             lhsT=hs[:, s * P:(s + 1) * P],
                                             rhs=ident_b[:], start=True,
                                             stop=True)
                        ts = ph0.tile([P, 512], bf16, tag="ts")
                        nc.vector.tensor_copy(ts[:], tp[:])
                        nc.sync.dma_start(
                            out=tab[j * 512:(j + 1) * 512, :].rearrange(
                                "(s p) o -> p s o", p=P),
                            in_=ts[:].rearrange("p (s o) -> p s o", s=4),
                        )

                build(table_hi, 0, hi_rows // 512)
                nc.sync.dma_start(out=table_hi[nhi:nhi + 1, :],
                                  in_=zrow[0:1, :])
                build(table_lo, hi_rows, lo_rows // 512)
                nc.sync.dma_start(out=table_lo[SPLIT:SPLIT + 1, :],
                                  in_=zrow[0:1, :])

        u1 = big.tile([P, ec], bf16)
        stats = consts.tile([P, nstat, 6], f32)

        def bn_coeffs(g_s, be_s, nchunk, corr=None):
            """Merge bn_stats 6-tuples -> AllReduce -> a, c (scale/bias)."""
            se = red.tile([P, nstat], f32, tag="se")
            nc.vector.tensor_mul(se[:, :nchunk], stats[:, :nchunk, 0],
                                 stats[:, :nchunk, 1])
            so = red.tile([P, nstat], f32, tag="so")
            nc.vector.tensor_mul(so[:, :nchunk], stats[:, :nchunk, 3],
                                 stats[:, :nchunk, 4])
            qe = red.tile([P, nstat], f32, tag="qe")
            nc.vector.tensor_mul(qe[:, :nchunk], se[:, :nchunk],
                                 stats[:, :nchunk, 1])
            nc.vector.tensor_add(qe[:, :nchunk], qe[:, :nchunk],
                                 stats[:, :nchunk, 2])
            qo = red.tile([P, nstat], f32, tag="qo")
            nc.vector.tensor_mul(qo[:, :nchunk], so[:, :nchunk],
                                 stats[:, :nchunk, 4])
            nc.vector.tensor_add(qo[:, :nchunk], qo[:, :nchunk],
                                 stats[:, :nchunk, 5])
            nc.vector.tensor_add(se[:, :nchunk], se[:, :nchunk], so[:, :nchunk])
            nc.vector.tensor_add(qe[:, :nchunk], qe[:, :nchunk], qo[:, :nchunk])
            sq = red.tile([P, 2], f32, tag="sq")
            nc.vector.tensor_reduce(sq[:, 0:1], se[:, :nchunk],
                                    axis=mybir.AxisListType.X,
                                    op=mybir.AluOpType.add)
            nc.vector.tensor_reduce(sq[:, 1:2], qe[:, :nchunk],
                                    axis=mybir.AxisListType.X,
                                    op=mybir.AluOpType.add)
            cc_in = dram.tile([P, 2], f32, tag="cc_in")
            nc.sync.dma_start(out=cc_in[:], in_=sq[:])
            cc_out = dram.tile([P, 2], f32, tag="cc_out")
            nc.gpsimd.collective_compute(
                "AllReduce", mybir.AluOpType.add, replica_groups=grp_all,
                ins=[cc_in[:].opt()], outs=[cc_out[:].opt()])
            sqg = red.tile([P, 2], f32, tag="sqg")
            nc.sync.dma_start(out=sqg[:], in_=cc_out[:])
            if corr is not None:
                # subtract the pad edges' (constant) contribution
                v, vq = corr
                t = red.tile([P, 2], f32, tag="tcorr")
                nc.vector.tensor_scalar_mul(t[:, 0:1], v[:], float(n_pad_tot))
                nc.vector.tensor_scalar_mul(t[:, 1:2], vq[:], float(n_pad_tot))
                nc.vector.tensor_sub(sqg[:], sqg[:], t[:])
            mu = red.tile([P, 1], f32, tag="mu")
            nc.vector.tensor_scalar_mul(mu[:], sqg[:, 0:1], 1.0 / e_total)
            var = red.tile([P, 1], f32, tag="var")
            nc.vector.tensor_scalar_mul(var[:], sqg[:, 1:2], 1.0 / e_total)
            mu2 = red.tile([P, 1], f32, tag="mu2")
            nc.vector.tensor_mul(mu2[:], mu[:], mu[:])
            nc.vector.tensor_sub(var[:], var[:], mu2[:])
            a = red.tile([P, 1], f32, tag="a")
            nc.scalar.activation(a[:], var[:], func=FT.Sqrt, bias=eps_s[:],
                                 scale=1.0)
            nc.vector.reciprocal(a[:], a[:])
            nc.vector.tensor_mul(a[:], a[:], g_s[:])
            c = red.tile([P, 1], f32, tag="c")
            nc.vector.tensor_mul(c[:], mu[:], a[:])
            nc.vector.tensor_sub(c[:], be_s[:], c[:])
            return a, c

        with (
            tc.tile_pool(name="psA", bufs=4, space="PSUM") as psA,
            tc.tile_pool(name="psS", bufs=1, space="PSUM") as psS,
            tc.tile_pool(name="ea", bufs=4) as eap,
            tc.tile_pool(name="gp", bufs=3) as gp,
            tc.tile_pool(name="gs", bufs=3) as gsp,
            tc.tile_pool(name="op", bufs=3) as op,
        ):
            # ---- pass A: u1 = W1b@eaT + hW[src] + hW[dst] ---------------
            g_tiles = {}
            for gi, (off, L, src_hi, dst_hi) in enumerate(groups):
                if gi in idx_pre:
                    si, di = idx_pre[gi]
                else:
                    si = gidx.tile([P, GROUP // 16], i16, tag="si")
                    nc.sync.dma_start(out=si[:, :L // 16],
                                      in_=sidx[:, off // 16:(off + L) // 16])
                    di = gidx.tile([P, GROUP // 16], i16, tag="di")
                    nc.sync.dma_start(out=di[:, :L // 16],
                                      in_=didx[:, off // 16:(off + L) // 16])
                gsrc = gp.tile([P, GROUP], bf16, tag="gsrc")
                gdst = gp.tile([P, GROUP], bf16, tag="gdst")
                src_base = table_hi[:, :] if src_hi else table_lo[:, :]
                dst_base = table_hi[:, :] if dst_hi else table_lo[:, :]
                nc.gpsimd.dma_gather(
                    out_ap=gsrc[:, :L].rearrange("p (a s) -> p a s", a=1),
                    in_ap=src_base, idxs_ap=si[:, :L // 16],
                    num_idxs=L, num_idxs_reg=L, elem_size=P,
                    transpose=True, single_packet=False)
                nc.gpsimd.dma_gather(
                    out_ap=gdst[:, :L].rearrange("p (a s) -> p a s", a=1),
                    in_ap=dst_base, idxs_ap=di[:, :L // 16],
                    num_idxs=L, num_idxs_reg=L, elem_size=P,
                    transpose=True, single_packet=False)
                g_tiles[gi] = (gsrc, gdst, off)

            for k, (off, S, gi) in enumerate(chunksA):
                gsrc, gdst, goff = g_tiles[gi]
                rel = off - goff
                ea_t = eap.tile([P, 512], bf16, tag="ea")
                nc.sync.dma_start(out=ea_t[:, :S], in_=eaT[:, off:off + S])
                up = psA.tile([P, 512], f32, tag="up")
                nc.tensor.matmul(up[:, :S], lhsT=w1bT[:], rhs=ea_t[:, :S],
                                 start=True, stop=True)
                gsum = gsp.tile([P, 512], bf16, tag="gsum")
                nc.vector.tensor_add(gsum[:, :S], gsrc[:, rel:rel + S],
                                     gdst[:, rel:rel + S])
                nc.vector.tensor_add(u1[:, off:off + S], up[:, :S],
                                     gsum[:, :S])
                nc.vector.bn_stats(stats[:, k, :], u1[:, off:off + S])

            a1, c1 = bn_coeffs(g1_s, be1_s, len(chunksA))

            # pad columns have u1 == 0 -> u2_pad = W2 @ relu(c1), constant
            rc = red.tile([P, 1], f32, tag="rc")
            nc.scalar.activation(rc[:], c1[:], func=FT.Relu)
            rcb = red.tile([P, 1], bf16, tag="rcb")
            nc.vector.tensor_copy(rcb[:], rc[:])
            vp = psS.tile([P, 1], f32, tag="vp")
            nc.tensor.matmul(vp[:], lhsT=w2T[:], rhs=rcb[:],
                             start=True, stop=True)
            v2 = red.tile([P, 1], f32, tag="v2")
            nc.vector.tensor_copy(v2[:], vp[:])
            v2q = red.tile([P, 1], f32, tag="v2q")
            nc.vector.tensor_mul(v2q[:], v2[:], v2[:])

            # ---- pass B: z1 = relu(a1*u1+c1) in place; stats of W2@z1 ---
            for k, (off, S) in enumerate(chunksBC):
                nc.scalar.activation(u1[:, off:off + S], u1[:, off:off + S],
                                     func=FT.Relu, scale=a1[:], bias=c1[:])
                up = psA.tile([P, 512], f32, tag="up")
                nc.tensor.matmul(up[:, :S], lhsT=w2T[:],
                                 rhs=u1[:, off:off + S], start=True, stop=True)
                nc.vector.bn_stats(stats[:, k, :], up[:, :S])

            a2, c2 = bn_coeffs(g2_s, be2_s, len(chunksBC), corr=(v2, v2q))

            # ---- pass C: out = relu(a2*(W2@z1)+c2) ----------------------
            for off, S in chunksBC:
                up = psA.tile([P, 512], f32, tag="up")
                nc.tensor.matmul(up[:, :S], lhsT=w2T[:],
                                 rhs=u1[:, off:off + S], start=True, stop=True)
                ot = op.tile([P, 512], bf16, tag="ot")
                nc.scalar.activation(ot[:, :S], up[:, :S], func=FT.Relu,
                                     scale=a2[:], bias=c2[:])
                nc.sync.dma_start(out=outT[:, off:off + S], in_=ot[:, :S])

    nc.compile()
    return nc


def _wrap16(a):
    """linear [L] -> [16, L/16] wrapped, tiled to [128, L/16]."""
    w = np.ascontiguousarray(a.reshape(-1, 16).T)
    return np.tile(w, (8, 1))


def host_prep(x, edge_index, edge_attr, n_cores):
    """Shard edges, bucket-sort, pad; returns per-core arrays + caps + pos."""
    n = x.shape[0]
    e = edge_attr.shape[0]
    ec0 = e // n_cores
    nhi, hi_rows, lo_rows, npad = table_layout(n)

    src_all = edge_index[0].astype(np.int64)
    dst_all = edge_index[1].astype(np.int64)

    per_core = []
    counts = np.zeros((n_cores, 4), np.int64)
    for c in range(n_cores):
        sl = slice(c * ec0, (c + 1) * ec0)
        s, d = src_all[sl], dst_all[sl]
        key = (s >= SPLIT) * 2 + (d >= SPLIT)
        order = np.argsort(key, kind="stable")
        counts[c] = np.bincount(key, minlength=4)
        per_core.append((s, d, key, order))

    caps = tuple(int(max(128, ((counts[:, b].max() + 127) // 128) * 128))
                 for b in range(4))
    ec = sum(caps)
    offs = {}
    _acc = 0
    for b in BUCKET_ORDER:
        offs[b] = _acc
        _acc += caps[b]

    zero_lo = SPLIT          # local zero-row idx in the lo region
    zero_hi = nhi            # local zero-row idx in the hi region

    cores = []
    for c in range(n_cores):
        s, d, key, order = per_core[c]
        cnt = counts[c]
        # padded position of each sorted edge
        pos_sorted = np.empty(ec0, np.int64)
        start = 0
        sidx_p = np.empty(ec, np.int64)
        didx_p = np.empty(ec, np.int64)
        ea_cols = np.full(ec, -1, np.int64)  # source edge for each padded col
        for b in range(4):
            idx_b = order[start:start + cnt[b]]
            pos = offs[b] + np.arange(cnt[b])
            pos_sorted[start:start + cnt[b]] = pos
            sb = s[idx_b]
            db = d[idx_b]
            src_hi, dst_hi = b >= 2, b % 2 == 1
            sl_loc = sb - SPLIT if src_hi else sb
            dl_loc = db - SPLIT if dst_hi else db
            sidx_p[pos] = sl_loc
            didx_p[pos] = dl_loc
            ea_cols[pos] = idx_b
            # pads
            padr = np.arange(offs[b] + cnt[b], offs[b] + caps[b])
            sidx_p[padr] = zero_hi if src_hi else zero_lo
            didx_p[padr] = zero_hi if dst_hi else zero_lo
            start += cnt[b]
        inv = np.empty(ec0, np.int64)
        inv[order] = pos_sorted  # padded position of original local edge
        cores.append((sidx_p.astype(np.int16), didx_p.astype(np.int16),
                      ea_cols, inv))
    return caps, ec, cores, npad


def make_in_maps(x, edge_index, edge_attr, W_lin, b_lin, W1, g1, be1, W2,
                 g2, be2, n_cores):
    n = x.shape[0]
    nhi, hi_rows, lo_rows, npad = table_layout(n)
    caps, ec, cores, _ = host_prep(x, edge_index, edge_attr, n_cores)

    # xT columns: [0, nhi) hi nodes, [hi_rows, hi_rows+SPLIT) lo nodes.
    xbf = x.astype(BF16)
    xT = np.zeros((P, npad), dtype=BF16)
    xT[:, 0:nhi] = xbf[SPLIT:n].T
    xT[:, hi_rows:hi_rows + SPLIT] = xbf[0:SPLIT].T

    f32c = np.ascontiguousarray
    wlin_h = f32c(W_lin.astype(np.float32))
    w1_h = f32c(W1.astype(np.float32))
    w2_h = f32c(W2.astype(np.float32))
    blin_h = f32c(b_lin.astype(np.float32).reshape(P, 1))
    g1_h = f32c(g1.astype(np.float32).reshape(P, 1))
    be1_h = f32c(be1.astype(np.float32).reshape(P, 1))
    g2_h = f32c(g2.astype(np.float32).reshape(P, 1))
    be2_h = f32c(be2.astype(np.float32).reshape(P, 1))

    groups, _ = edge_layout(caps)
    eabf = edge_attr.astype(BF16)

    in_maps = []
    invs = []
    for c in range(n_cores):
        sidx_p, didx_p, ea_cols, inv = cores[c]
        ec0 = inv.shape[0]
        eaT = np.zeros((P, ec), dtype=BF16)
        real = ea_cols >= 0
        eaT[:, real] = eabf[c * ec0 + ea_cols[real]].T
        sw = np.zeros((P, ec // 16), np.int16)
        dw = np.zeros((P, ec // 16), np.int16)
        for off, L, _, _ in groups:
            sw[:, off // 16:(off + L) // 16] = _wrap16(sidx_p[off:off + L])
            dw[:, off // 16:(off + L) // 16] = _wrap16(didx_p[off:off + L])
        in_maps.append({
            "eaT": eaT, "xT": xT, "sidx": sw, "didx": dw,
            "wlin": wlin_h, "w1": w1_h, "w2": w2_h, "blin": blin_h,
            "g1": g1_h, "be1": be1_h, "g2": g2_h, "be2": be2_h,
        })
        invs.append(inv)
    return caps, ec, in_maps, invs


_GRAPH_CACHE = {}


def get_graph(n_cores, caps, n_nodes, e_total):
    key = (n_cores, caps, n_nodes, e_total)
    if key not in _GRAPH_CACHE:
        _GRAPH_CACHE[key] = build_graph(n_cores, caps, n_nodes, e_total)
    return _GRAPH_CACHE[key]


def kernel(x, edge_index, edge_attr, W_lin, b_lin, W1, b1, g1, be1, W2, b2,
           g2, be2):
    """Full-input entry point: shard, run on 8 NeuronCores, gather."""
    x = np.asarray(x)
    edge_index = np.asarray(edge_index)
    edge_attr = np.asarray(edge_attr)
    e = edge_attr.shape[0]
    n = x.shape[0]
    ec0 = e // N_CORES

    caps, ec, in_maps, invs = make_in_maps(
        x, edge_index, edge_attr, np.asarray(W_lin), np.asarray(b_lin),
        np.asarray(W1), np.asarray(g1), np.asarray(be1), np.asarray(W2),
        np.asarray(g2), np.asarray(be2), N_CORES)
    nc = get_graph(N_CORES, caps, n, e)
    res = run_bass_kernel_spmd(nc, in_maps, core_ids=list(range(N_CORES)))
    out = np.empty((e, NIN), dtype=np.float32)
    for c in range(N_CORES):
        oT = np.asarray(res.results[c]["outT"], dtype=np.float32)
        out[c * ec0:(c + 1) * ec0] = oT.T[invs[c]]
    return out



# revision 6
# speedup vs baseline: 1.3636x; 1.3636x over previous
"""Trainium2 Bass kernel for the GNN edge-update MLP (8 NeuronCores).

Reference semantics:
    h   = x @ W_lin.T + b_lin                       # [N, nin]
    agg = h[src] + h[dst]                           # [E, nin]
    z   = concat([agg, edge_attr], -1)              # [E, 2*nin]
    z   = relu(BN(z @ W1.T + b1; g1, be1))          # [E, nout]  (BN over edges)
    z   = relu(BN(z @ W2.T + b2; g2, be2))          # [E, nout]
    (training-mode BN: batch statistics over the edge axis)

Restructuring:
  * b1/b2 cancel inside training-mode BN -> dropped.  So does b_lin:
    W1a @ (2 b_lin) is a per-feature constant across edges and BN is
    mean-subtracted, so it cancels too.
  * z @ W1.T = Wc@(x[src]+x[dst]) + W1b@eaT + const, with W1 = [W1a | W1b]
    and Wc = W1a @ W_lin.  x is gathered DIRECTLY (node-major rows in DRAM,
    no precomputed table, no serial table-build phase); Wc is folded in
    AFTER aggregation as a second accumulating matmul.
  * Everything on device is feature-major [128, edges]; host pre-transposes
    edge_attr and post-transposes the output.
  * Gathers use GPSIMD dma_gather(transpose=True): int16 indices (signed on
    HW), so x is staged in two DRAM regions (hi nodes first, then lo) and
    the host bucket-sorts each core's edges by (src>=SPLIT, dst>=SPLIT) so
    every gather instruction targets one region with small non-negative
    local indices.  Buckets are padded (to the max size over cores) with
    edges that gather dedicated zero rows and have zero edge_attr, so padded
    u1 columns are exactly 0; their (constant) effect on the second BN's
    statistics is subtracted analytically on device.
  * BN statistics: per-chunk vector bn_stats, merged manually, AllReduce'd
    across the 8 cores ([128,2] f32 - tiny).
"""

import sys
from contextlib import ExitStack

import numpy as np

try:
    import concourse  # noqa: F401
except ImportError:  # pragma: no cover
    sys.path.insert(0, "/opt/trn_rl_repo")

import ml_dtypes
from concourse import bass, bacc, mybir
from concourse import tile
from concourse.bass_utils import run_bass_kernel_spmd
from concourse.masks import make_identity

BF16 = ml_dtypes.bfloat16

N_CORES = 8
NIN = 128
EPS = 1e-5
P = 128

SPLIT = 32767            # nodes < SPLIT are "lo", >= SPLIT are "hi"
BUCKET_ORDER = (3, 1, 2, 0)
GROUP = 2048             # edges per dma_gather instruction
NQ = 1                   # SWDGE queues (multi-queue corrupts: shared rings)


def table_layout(n_nodes):
    """Two gather regions of x rows: hi = x[SPLIT:] + zero row (padded to
    512); lo = x[0:SPLIT) + zero row.  xp rows: [hi | lo] in that order."""
    nhi = n_nodes - SPLIT
    hi_rows = ((nhi + 1 + 511) // 512) * 512
    lo_rows = ((SPLIT + 1 + 511) // 512) * 512
    npad = hi_rows + lo_rows
    return nhi, hi_rows, lo_rows, npad


def edge_layout(caps):
    """Device-side loop structure from bucket capacities.

    Returns (groups, chunks): groups = (off, L, src_hi, dst_hi);
    chunks = (off, S, group_index)."""
    groups = []
    chunks = []
    off = 0
    for b in BUCKET_ORDER:
        src_hi, dst_hi = b >= 2, b % 2 == 1
        rem = caps[b]
        while rem > 0:
            L = min(GROUP, rem)
            gi = len(groups)
            groups.append((off, L, src_hi, dst_hi))
            coff = 0
            while coff < L:
                S = min(512, L - coff)
                chunks.append((off + coff, S, gi))
                coff += S
            off += L
            rem -= L
    return groups, chunks


def _chunks(ec):
    out = []
    off = 0
    while off < ec:
        s = min(512, ec - off)
        out.append((off, s))
        off += s
    return out


def build_graph(n_cores, caps, n_nodes, e_total, eps=EPS):
    f32 = mybir.dt.float32
    bf16 = mybir.dt.bfloat16
    i16 = mybir.dt.int16
    FT = mybir.ActivationFunctionType

    nc = bacc.Bacc(
        "TRN2", target_bir_lowering=False, debug=False, num_devices=n_cores,
        num_swdge_queues=NQ,
    )

    nhi, hi_rows, lo_rows, npad = table_layout(n_nodes)
    groups, chunksA = edge_layout(caps)
    ec = sum(caps)
    chunksBC = _chunks(ec)
    nstat = max(len(chunksA), len(chunksBC))
    n_pad_tot = ec * n_cores - e_total  # padded edges across all cores

    # ---- I/O -------------------------------------------------------------
    eaT = nc.dram_tensor("eaT", [P, ec], bf16, kind="ExternalInput").ap()
    xp = nc.dram_tensor("xp", [npad, P], bf16, kind="ExternalInput").ap()
    sidx = nc.dram_tensor("sidx", [P, ec // 16], i16, kind="ExternalInput").ap()
    didx = nc.dram_tensor("didx", [P, ec // 16], i16, kind="ExternalInput").ap()
    wlin = nc.dram_tensor("wlin", [P, P], f32, kind="ExternalInput").ap()
    w1 = nc.dram_tensor("w1", [P, 2 * P], f32, kind="ExternalInput").ap()
    w2 = nc.dram_tensor("w2", [P, P], f32, kind="ExternalInput").ap()
    g1 = nc.dram_tensor("g1", [P, 1], f32, kind="ExternalInput").ap()
    be1 = nc.dram_tensor("be1", [P, 1], f32, kind="ExternalInput").ap()
    g2 = nc.dram_tensor("g2", [P, 1], f32, kind="ExternalInput").ap()
    be2 = nc.dram_tensor("be2", [P, 1], f32, kind="ExternalInput").ap()
    outT = nc.dram_tensor("outT", [P, ec], bf16, kind="ExternalOutput").ap()

    grp_all = [list(range(n_cores))]

    with tile.TileContext(nc) as tc, ExitStack() as es:
        consts = es.enter_context(tc.tile_pool(name="consts", bufs=1))
        gidx = es.enter_context(tc.tile_pool(name="gidx", bufs=4))
        dram = es.enter_context(tc.tile_pool(name="dram", bufs=1, space="DRAM"))
        big = es.enter_context(tc.tile_pool(name="big", bufs=1))
        red = es.enter_context(tc.tile_pool(name="red", bufs=1))

        # ---- constants / weight prep ------------------------------------
        ident_f = consts.tile([P, P], f32)
        make_identity(nc, ident_f[:])

        wlin_s = consts.tile([P, P], f32)
        nc.sync.dma_start(out=wlin_s[:], in_=wlin)
        w1_s = consts.tile([P, 2 * P], f32)
        nc.sync.dma_start(out=w1_s[:], in_=w1)
        w2_s = consts.tile([P, P], f32)
        nc.sync.dma_start(out=w2_s[:], in_=w2)
        g1_s = consts.tile([P, 1], f32)
        nc.sync.dma_start(out=g1_s[:], in_=g1)
        be1_s = consts.tile([P, 1], f32)
        nc.sync.dma_start(out=be1_s[:], in_=be1)
        g2_s = consts.tile([P, 1], f32)
        nc.sync.dma_start(out=g2_s[:], in_=g2)
        be2_s = consts.tile([P, 1], f32)
        nc.sync.dma_start(out=be2_s[:], in_=be2)
        eps_s = consts.tile([P, 1], f32)
        nc.vector.memset(eps_s[:], eps)

        # preload gather indices for the first groups so gathers can start
        # before the (tiny) weight-prep finishes
        idx_pre = {}
        for gi, (off, L, _sh, _dh) in enumerate(groups[:4]):
            si = gidx.tile([P, GROUP // 16], i16, tag="si")
            nc.sync.dma_start(out=si[:, :L // 16],
                              in_=sidx[:, off // 16:(off + L) // 16])
            di = gidx.tile([P, GROUP // 16], i16, tag="di")
            nc.sync.dma_start(out=di[:, :L // 16],
                              in_=didx[:, off // 16:(off + L) // 16])
            idx_pre[gi] = (si, di)

        w1aT = consts.tile([P, P], f32)
        w1bT = consts.tile([P, P], bf16)
        w2T = consts.tile([P, P], bf16)
        wcT = consts.tile([P, P], bf16)

        with tc.tile_pool(name="psum0", bufs=1, space="PSUM") as psw:
            pw = psw.tile([P, P], f32, tag="pw")
            nc.tensor.matmul(pw[:], lhsT=w1_s[:, 0:P], rhs=ident_f[:],
                             start=True, stop=True)
            nc.vector.tensor_copy(w1aT[:], pw[:])
            pw = psw.tile([P, P], f32, tag="pw")
            nc.tensor.matmul(pw[:], lhsT=w1_s[:, P:2 * P], rhs=ident_f[:],
                             start=True, stop=True)
            nc.vector.tensor_copy(w1bT[:], pw[:])
            pw = psw.tile([P, P], f32, tag="pw")
            nc.tensor.matmul(pw[:], lhsT=w2_s[:], rhs=ident_f[:],
                             start=True, stop=True)
            nc.vector.tensor_copy(w2T[:], pw[:])
            # WcT[i, o] = (W1a @ W_lin)[o, i]
            pw = psw.tile([P, P], f32, tag="pw")
            nc.tensor.matmul(pw[:], lhsT=wlin_s[:], rhs=w1aT[:],
                             start=True, stop=True)
            nc.vector.tensor_copy(wcT[:], pw[:])

        u1 = big.tile([P, ec], bf16)
        stats = consts.tile([P, nstat, 6], f32)

        def bn_coeffs(g_s, be_s, nchunk, corr=None):
            """Merge bn_stats 6-tuples -> AllReduce -> a, c (scale/bias)."""
            se = red.tile([P, nstat], f32, tag="se")
            nc.vector.tensor_mul(se[:, :nchunk], stats[:, :nchunk, 0],
                                 stats[:, :nchunk, 1])
            so = red.tile([P, nstat], f32, tag="so")
            nc.vector.tensor_mul(so[:, :nchunk], stats[:, :nchunk, 3],
                                 stats[:, :nchunk, 4])
            qe = red.tile([P, nstat], f32, tag="qe")
            nc.vector.tensor_mul(qe[:, :nchunk], se[:, :nchunk],
                                 stats[:, :nchunk, 1])
            nc.vector.tensor_add(qe[:, :nchunk], qe[:, :nchunk],
                                 stats[:, :nchunk, 2])
            qo = red.tile([P, nstat], f32, tag="qo")
            nc.vector.tensor_mul(qo[:, :nchunk], so[:, :nchunk],
                                 stats[:, :nchunk, 4])
            nc.vector.tensor_add(qo[:, :nchunk], qo[:, :nchunk],
                                 stats[:, :nchunk, 5])
            nc.vector.tensor_add(se[:, :nchunk], se[:, :nchunk], so[:, :nchunk])
            nc.vector.tensor_add(qe[:, :nchunk], qe[:, :nchunk], qo[:, :nchunk])
            sq = red.tile([P, 2], f32, tag="sq")
            nc.vector.tensor_reduce(sq[:, 0:1], se[:, :nchunk],
                                    axis=mybir.AxisListType.X,
                                    op=mybir.AluOpType.add)
            nc.vector.tensor_reduce(sq[:, 1:2], qe[:, :nchunk],
                                    axis=mybir.AxisListType.X,
                                    op=mybir.AluOpType.add)
            cc_in = dram.tile([P, 2], f32, tag="cc_in")
            nc.sync.dma_start(out=cc_in[:], in_=sq[:])
            cc_out = dram.tile([P, 2], f32, tag="cc_out")
            nc.gpsimd.collective_compute(
                "AllReduce", mybir.AluOpType.add, replica_groups=grp_all,
                ins=[cc_in[:].opt()], outs=[cc_out[:].opt()])
            sqg = red.tile([P, 2], f32, tag="sqg")
            nc.sync.dma_start(out=sqg[:], in_=cc_out[:])
            if corr is not None:
                # subtract the pad edges' (constant) contribution
                v, vq = corr
                t = red.tile([P, 2], f32, tag="tcorr")
                nc.vector.tensor_scalar_mul(t[:, 0:1], v[:], float(n_pad_tot))
                nc.vector.tensor_scalar_mul(t[:, 1:2], vq[:], float(n_pad_tot))
                nc.vector.tensor_sub(sqg[:], sqg[:], t[:])
            mu = red.tile([P, 1], f32, tag="mu")
            nc.vector.tensor_scalar_mul(mu[:], sqg[:, 0:1], 1.0 / e_total)
            var = red.tile([P, 1], f32, tag="var")
            nc.vector.tensor_scalar_mul(var[:], sqg[:, 1:2], 1.0 / e_total)
            mu2 = red.tile([P, 1], f32, tag="mu2")
            nc.vector.tensor_mul(mu2[:], mu[:], mu[:])
            nc.vector.tensor_sub(var[:], var[:], mu2[:])
            a = red.tile([P, 1], f32, tag="a")
            nc.scalar.activation(a[:], var[:], func=FT.Sqrt, bias=eps_s[:],
                                 scale=1.0)
            nc.vector.reciprocal(a[:], a[:])
            nc.vector.tensor_mul(a[:], a[:], g_s[:])
            c = red.tile([P, 1], f32, tag="c")
            nc.vector.tensor_mul(c[:], mu[:], a[:])
            nc.vector.tensor_sub(c[:], be_s[:], c[:])
            return a, c

        with (
            tc.tile_pool(name="psA", bufs=4, space="PSUM") as psA,
            tc.tile_pool(name="psS", bufs=1, space="PSUM") as psS,
            tc.tile_pool(name="ea", bufs=4) as eap,
            tc.tile_pool(name="gp", bufs=3) as gp,
            tc.tile_pool(name="gs", bufs=3) as gsp,
            tc.tile_pool(name="op", bufs=3) as op,
        ):
            # ---- pass A: u1 = Wc@(x[src]+x[dst]) + W1b@eaT ---------------
            g_tiles = {}
            for gi, (off, L, src_hi, dst_hi) in enumerate(groups):
                if gi in idx_pre:
                    si, di = idx_pre[gi]
                else:
                    si = gidx.tile([P, GROUP // 16], i16, tag="si")
                    nc.sync.dma_start(out=si[:, :L // 16],
                                      in_=sidx[:, off // 16:(off + L) // 16])
                    di = gidx.tile([P, GROUP // 16], i16, tag="di")
                    nc.sync.dma_start(out=di[:, :L // 16],
                                      in_=didx[:, off // 16:(off + L) // 16])
                gsrc = gp.tile([P, GROUP], bf16, tag="gsrc")
                gdst = gp.tile([P, GROUP], bf16, tag="gdst")
                src_base = xp[0:hi_rows, :] if src_hi else xp[hi_rows:npad, :]
                dst_base = xp[0:hi_rows, :] if dst_hi else xp[hi_rows:npad, :]
                nc.gpsimd.dma_gather(
                    out_ap=gsrc[:, :L].rearrange("p (a s) -> p a s", a=1),
                    in_ap=src_base, idxs_ap=si[:, :L // 16],
                    num_idxs=L, num_idxs_reg=L, elem_size=P,
                    transpose=True, single_packet=False,
                    queue_num=(2 * gi) % NQ)
                nc.gpsimd.dma_gather(
                    out_ap=gdst[:, :L].rearrange("p (a s) -> p a s", a=1),
                    in_ap=dst_base, idxs_ap=di[:, :L // 16],
                    num_idxs=L, num_idxs_reg=L, elem_size=P,
                    transpose=True, single_packet=False,
                    queue_num=(2 * gi + 1) % NQ)
                g_tiles[gi] = (gsrc, gdst, off)

            for k, (off, S, gi) in enumerate(chunksA):
                gsrc, gdst, goff = g_tiles[gi]
                rel = off - goff
                ea_t = eap.tile([P, 512], bf16, tag="ea")
                nc.sync.dma_start(out=ea_t[:, :S], in_=eaT[:, off:off + S])
                gsum = gsp.tile([P, 512], bf16, tag="gsum")
                nc.vector.tensor_add(gsum[:, :S], gsrc[:, rel:rel + S],
                                     gdst[:, rel:rel + S])
                up = psA.tile([P, 512], f32, tag="up")
                nc.tensor.matmul(up[:, :S], lhsT=wcT[:], rhs=gsum[:, :S],
                                 start=True, stop=False)
                nc.tensor.matmul(up[:, :S], lhsT=w1bT[:], rhs=ea_t[:, :S],
                                 start=False, stop=True)
                nc.scalar.activation(u1[:, off:off + S], up[:, :S],
                                     func=FT.Identity, scale=1.0)
                nc.vector.bn_stats(stats[:, k, :], u1[:, off:off + S])

            a1, c1 = bn_coeffs(g1_s, be1_s, len(chunksA))

            # pad columns have u1 == 0 -> u2_pad = W2 @ relu(c1), constant
            rc = red.tile([P, 1], f32, tag="rc")
            nc.scalar.activation(rc[:], c1[:], func=FT.Relu)
            rcb = red.tile([P, 1], bf16, tag="rcb")
            nc.vector.tensor_copy(rcb[:], rc[:])
            vp = psS.tile([P, 1], f32, tag="vp")
            nc.tensor.matmul(vp[:], lhsT=w2T[:], rhs=rcb[:],
                             start=True, stop=True)
            v2 = red.tile([P, 1], f32, tag="v2")
            nc.vector.tensor_copy(v2[:], vp[:])
            v2q = red.tile([P, 1], f32, tag="v2q")
            nc.vector.tensor_mul(v2q[:], v2[:], v2[:])

            # ---- pass B: z1 = relu(a1*u1+c1) in place; stats of W2@z1 ---
            for k, (off, S) in enumerate(chunksBC):
                nc.scalar.activation(u1[:, off:off + S], u1[:, off:off + S],
                                     func=FT.Relu, scale=a1[:], bias=c1[:])
                up = psA.tile([P, 512], f32, tag="up")
                nc.tensor.matmul(up[:, :S], lhsT=w2T[:],
                                 rhs=u1[:, off:off + S], start=True, stop=True)
                nc.vector.bn_stats(stats[:, k, :], up[:, :S])

            a2, c2 = bn_coeffs(g2_s, be2_s, len(chunksBC), corr=(v2, v2q))

            # ---- pass C: out = relu(a2*(W2@z1)+c2) ----------------------
            for off, S in chunksBC:
                up = psA.tile([P, 512], f32, tag="up")
                nc.tensor.matmul(up[:, :S], lhsT=w2T[:],
                                 rhs=u1[:, off:off + S], start=True, stop=True)
                ot = op.tile([P, 512], bf16, tag="ot")
                nc.scalar.activation(ot[:, :S], up[:, :S], func=FT.Relu,
                                     scale=a2[:], bias=c2[:])
                nc.sync.dma_start(out=outT[:, off:off + S], in_=ot[:, :S])

    nc.compile()
    return nc


def _wrap16(a):
    """linear [L] -> [16, L/16] wrapped, tiled to [128, L/16]."""
    w = np.ascontiguousarray(a.reshape(-1, 16).T)
    return np.tile(w, (8, 1))


def host_prep(x, edge_index, edge_attr, n_cores):
    """Shard edges, bucket-sort, pad; returns per-core arrays + caps + pos."""
    n = x.shape[0]
    e = edge_attr.shape[0]
    ec0 = e // n_cores
    nhi, hi_rows, lo_rows, npad = table_layout(n)

    src_all = edge_index[0].astype(np.int64)
    dst_all = edge_index[1].astype(np.int64)

    per_core = []
    counts = np.zeros((n_cores, 4), np.int64)
    for c in range(n_cores):
        sl = slice(c * ec0, (c + 1) * ec0)
        s, d = src_all[sl], dst_all[sl]
        key = (s >= SPLIT) * 2 + (d >= SPLIT)
        order = np.argsort(key, kind="stable")
        counts[c] = np.bincount(key, minlength=4)
        per_core.append((s, d, key, order))

    caps = tuple(int(max(128, ((counts[:, b].max() + 127) // 128) * 128))
                 for b in range(4))
    ec = sum(caps)
    offs = {}
    _acc = 0
    for b in BUCKET_ORDER:
        offs[b] = _acc
        _acc += caps[b]

    zero_lo = SPLIT          # local zero-row idx in the lo region
    zero_hi = nhi            # local zero-row idx in the hi region

    cores = []
    for c in range(n_cores):
        s, d, key, order = per_core[c]
        cnt = counts[c]
        # padded position of each sorted edge
        pos_sorted = np.empty(ec0, np.int64)
        start = 0
        sidx_p = np.empty(ec, np.int64)
        didx_p = np.empty(ec, np.int64)
        ea_cols = np.full(ec, -1, np.int64)  # source edge for each padded col
        for b in range(4):
            idx_b = order[start:start + cnt[b]]
            pos = offs[b] + np.arange(cnt[b])
            pos_sorted[start:start + cnt[b]] = pos
            sb = s[idx_b]
            db = d[idx_b]
            src_hi, dst_hi = b >= 2, b % 2 == 1
            sl_loc = sb - SPLIT if src_hi else sb
            dl_loc = db - SPLIT if dst_hi else db
            sidx_p[pos] = sl_loc
            didx_p[pos] = dl_loc
            ea_cols[pos] = idx_b
            # pads
            padr = np.arange(offs[b] + cnt[b], offs[b] + caps[b])
            sidx_p[padr] = zero_hi if src_hi else zero_lo
            didx_p[padr] = zero_hi if dst_hi else zero_lo
            start += cnt[b]
        inv = np.empty(ec0, np.int64)
        inv[order] = pos_sorted  # padded position of original local edge
        cores.append((sidx_p.astype(np.int16), didx_p.astype(np.int16),
                      ea_cols, inv))
    return caps, ec, cores, npad


def make_in_maps(x, edge_index, edge_attr, W_lin, b_lin, W1, g1, be1, W2,
                 g2, be2, n_cores):
    n = x.shape[0]
    nhi, hi_rows, lo_rows, npad = table_layout(n)
    caps, ec, cores, _ = host_prep(x, edge_index, edge_attr, n_cores)

    # xp rows: [0, nhi) hi nodes, [hi_rows, hi_rows+SPLIT) lo nodes;
    # rows nhi and hi_rows+SPLIT are the zero rows pads gather.
    xbf = np.asarray(x).astype(BF16)
    xp = np.zeros((npad, P), dtype=BF16)
    xp[0:nhi] = xbf[SPLIT:n]
    xp[hi_rows:hi_rows + SPLIT] = xbf[0:SPLIT]

    f32c = np.ascontiguousarray
    wlin_h = f32c(W_lin.astype(np.float32))
    w1_h = f32c(W1.astype(np.float32))
    w2_h = f32c(W2.astype(np.float32))
    g1_h = f32c(g1.astype(np.float32).reshape(P, 1))
    be1_h = f32c(be1.astype(np.float32).reshape(P, 1))
    g2_h = f32c(g2.astype(np.float32).reshape(P, 1))
    be2_h = f32c(be2.astype(np.float32).reshape(P, 1))

    groups, _ = edge_layout(caps)
    eabf = np.asarray(edge_attr).astype(BF16)

    in_maps = []
    invs = []
    for c in range(n_cores):
        sidx_p, didx_p, ea_cols, inv = cores[c]
        ec0 = inv.shape[0]
        eaT = np.zeros((P, ec), dtype=BF16)
        real = ea_cols >= 0
        eaT[:, real] = eabf[c * ec0 + ea_cols[real]].T
        sw = np.zeros((P, ec // 16), np.int16)
        dw = np.zeros((P, ec // 16), np.int16)
        for off, L, _, _ in groups:
            sw[:, off // 16:(off + L) // 16] = _wrap16(sidx_p[off:off + L])
            dw[:, off // 16:(off + L) // 16] = _wrap16(didx_p[off:off + L])
        in_maps.append({
            "eaT": eaT, "xp": xp, "sidx": sw, "didx": dw,
            "wlin": wlin_h, "w1": w1_h, "w2": w2_h,
            "g1": g1_h, "be1": be1_h, "g2": g2_h, "be2": be2_h,
        })
        invs.append(inv)
    return caps, ec, in_maps, invs


_GRAPH_CACHE = {}


def get_graph(n_cores, caps, n_nodes, e_total):
    key = (n_cores, caps, n_nodes, e_total)
    if key not in _GRAPH_CACHE:
        _GRAPH_CACHE[key] = build_graph(n_cores, caps, n_nodes, e_total)
    return _GRAPH_CACHE[key]


def kernel(x, edge_index, edge_attr, W_lin, b_lin, W1, b1, g1, be1, W2, b2,
           g2, be2):
    """Full-input entry point: shard, run on 8 NeuronCores, gather."""
    x = np.asarray(x)
    edge_index = np.asarray(edge_index)
    edge_attr = np.asarray(edge_attr)
    e = edge_attr.shape[0]
    n = x.shape[0]
    ec0 = e // N_CORES

    caps, ec, in_maps, invs = make_in_maps(
        x, edge_index, edge_attr, np.asarray(W_lin), np.asarray(b_lin),
        np.asarray(W1), np.asarray(g1), np.asarray(be1), np.asarray(W2),
        np.asarray(g2), np.asarray(be2), N_CORES)
    nc = get_graph(N_CORES, caps, n, e)
    res = run_bass_kernel_spmd(nc, in_maps, core_ids=list(range(N_CORES)))
    out = np.empty((e, NIN), dtype=np.float32)
    for c in range(N_CORES):
        oT = np.asarray(res.results[c]["outT"], dtype=np.float32)
        out[c * ec0:(c + 1) * ec0] = oT.T[invs[c]]
    return out


# revision 7
# speedup vs baseline: 1.6701x; 1.2247x over previous
"""Trainium2 Bass kernel v3: src-sharded edges; src side via window-expand
matmuls (zero gather descriptors), dst side via dma_gather.

Math (training-mode BN makes all constant per-feature shifts cancel):
    u1  = Wc@x[src] + Wc@x[dst] + W1b@eaT      (Wc = W1a @ W_lin)
    z1  = relu(a1*u1 + c1)                      (BN1 coeffs from global stats)
    out = relu(a2*(W2@z1) + c2)                 (BN2 coeffs from global stats)

Sharding: core c owns edges with src in [c*NPC, (c+1)*NPC).  Within a core,
edges are bucketed by dst >= SPLIT (int16 gather regions) and sorted by src;
512-edge chunks are packed so each chunk's src values span < 256 nodes.  The
host stages each chunk's 256-node x window (feature-major) in `xwins`; the
device rebuilds x[src] per chunk with two one-hot expand matmuls:
    onehot[n, j] = (swin[j] == n),  swin = src - window_base (fp16, -1 = pad)
    built as:  repl = ones1.T @ swin (1-partition matmul), then DVE is_equal
    against an iota column.
Only x[dst] is fetched with dma_gather (the per-descriptor generation rate on
the GPSIMD Q7 core, ~8 ns/row, is the whole kernel's bottleneck — the src
side's descriptors are eliminated entirely).
Pads gather dedicated zero rows and have zero edge_attr and swin=-1, so padded
u1 columns are exactly 0; their effect on BN2's statistics is subtracted
analytically.  BN statistics: per-chunk bn_stats, merged, AllReduce'd.
"""

import sys
from contextlib import ExitStack

import numpy as np

try:
    import concourse  # noqa: F401
except ImportError:  # pragma: no cover
    sys.path.insert(0, "/opt/trn_rl_repo")

import ml_dtypes
from concourse import bass, bacc, mybir
from concourse import tile
from concourse.bass_utils import run_bass_kernel_spmd
from concourse.masks import make_identity

BF16 = ml_dtypes.bfloat16

N_CORES = 8
NIN = 128
EPS = 1e-5
P = 128

NPC = 6272               # src nodes per core (49 * 128)
SPLIT = 32767            # dst nodes < SPLIT are "lo", >= SPLIT are "hi"
BUCKET_ORDER = (1, 0)    # dst-hi bucket first, then dst-lo
GROUP = 2048             # edges per dst dma_gather instruction
WSPAN = 256              # max src span per 512-edge chunk (2 x 128 windows)
CHUNK = 512


def table_layout(n_nodes):
    """dst gather regions: hi = x[SPLIT:] + zero row; lo = x[0:SPLIT) + zero."""
    nhi = n_nodes - SPLIT
    hi_rows = ((nhi + 1 + 511) // 512) * 512
    lo_rows = ((SPLIT + 1 + 511) // 512) * 512
    npad = hi_rows + lo_rows
    return nhi, hi_rows, lo_rows, npad


def edge_layout(caps):
    """groups = (off, L, dst_hi); chunks = (off, gi).  caps are %512."""
    groups = []
    chunks = []
    off = 0
    for b in BUCKET_ORDER:
        dst_hi = b == 1
        rem = caps[b]
        while rem > 0:
            L = min(GROUP, rem)
            gi = len(groups)
            groups.append((off, L, dst_hi))
            for coff in range(0, L, CHUNK):
                chunks.append((off + coff, gi))
            off += L
            rem -= L
    return groups, chunks


def build_graph(n_cores, caps, n_nodes, e_total, eps=EPS):
    f32 = mybir.dt.float32
    bf16 = mybir.dt.bfloat16
    f16 = mybir.dt.float16
    i16 = mybir.dt.int16
    i32 = mybir.dt.int32
    FT = mybir.ActivationFunctionType

    nc = bacc.Bacc(
        "TRN2", target_bir_lowering=False, debug=False, num_devices=n_cores,
    )

    nhi, hi_rows, lo_rows, npad = table_layout(n_nodes)
    groups, chunksA = edge_layout(caps)
    ec = sum(caps)
    nchunk = ec // CHUNK
    n_pad_tot = ec * n_cores - e_total

    # ---- I/O -------------------------------------------------------------
    eaT = nc.dram_tensor("eaT", [P, ec], bf16, kind="ExternalInput").ap()
    xp = nc.dram_tensor("xp", [npad, P], bf16, kind="ExternalInput").ap()
    xwins = nc.dram_tensor("xwins", [P, ec // 2], bf16,
                           kind="ExternalInput").ap()
    swin = nc.dram_tensor("swin", [1, ec], f16, kind="ExternalInput").ap()
    didx = nc.dram_tensor("didx", [P, ec // 16], i16, kind="ExternalInput").ap()
    wlin = nc.dram_tensor("wlin", [P, P], f32, kind="ExternalInput").ap()
    w1 = nc.dram_tensor("w1", [P, 2 * P], f32, kind="ExternalInput").ap()
    w2 = nc.dram_tensor("w2", [P, P], f32, kind="ExternalInput").ap()
    g1 = nc.dram_tensor("g1", [P, 1], f32, kind="ExternalInput").ap()
    be1 = nc.dram_tensor("be1", [P, 1], f32, kind="ExternalInput").ap()
    g2 = nc.dram_tensor("g2", [P, 1], f32, kind="ExternalInput").ap()
    be2 = nc.dram_tensor("be2", [P, 1], f32, kind="ExternalInput").ap()
    outT = nc.dram_tensor("outT", [P, ec], bf16, kind="ExternalOutput").ap()

    grp_all = [list(range(n_cores))]

    with tile.TileContext(nc) as tc, ExitStack() as es:
        consts = es.enter_context(tc.tile_pool(name="consts", bufs=1))
        gidx = es.enter_context(tc.tile_pool(name="gidx", bufs=4))
        dram = es.enter_context(tc.tile_pool(name="dram", bufs=1, space="DRAM"))
        big = es.enter_context(tc.tile_pool(name="big", bufs=1))
        red = es.enter_context(tc.tile_pool(name="red", bufs=1))

        # ---- constants / weight prep ------------------------------------
        ident_f = consts.tile([P, P], f32)
        make_identity(nc, ident_f[:])

        wlin_s = consts.tile([P, P], f32)
        nc.sync.dma_start(out=wlin_s[:], in_=wlin)
        w1_s = consts.tile([P, 2 * P], f32)
        nc.sync.dma_start(out=w1_s[:], in_=w1)
        w2_s = consts.tile([P, P], f32)
        nc.sync.dma_start(out=w2_s[:], in_=w2)
        g1_s = consts.tile([P, 1], f32)
        nc.sync.dma_start(out=g1_s[:], in_=g1)
        be1_s = consts.tile([P, 1], f32)
        nc.sync.dma_start(out=be1_s[:], in_=be1)
        g2_s = consts.tile([P, 1], f32)
        nc.sync.dma_start(out=g2_s[:], in_=g2)
        be2_s = consts.tile([P, 1], f32)
        nc.sync.dma_start(out=be2_s[:], in_=be2)
        eps_s = consts.tile([P, 1], f32)
        nc.vector.memset(eps_s[:], eps)
        ones1 = consts.tile([1, P], f16)
        nc.vector.memset(ones1[:], 1.0)
        iota_i = consts.tile([P, 1], i32)
        nc.gpsimd.iota(iota_i[:], pattern=[[0, 1]], base=0,
                       channel_multiplier=1)
        iota_f = consts.tile([P, 1], f32)
        nc.vector.tensor_copy(iota_f[:], iota_i[:])

        # preload dst gather indices for the first groups
        idx_pre = {}
        for gi, (off, L, _dh) in enumerate(groups[:4]):
            di = gidx.tile([P, GROUP // 16], i16, tag="di")
            nc.sync.dma_start(out=di[:, :L // 16],
                              in_=didx[:, off // 16:(off + L) // 16])
            idx_pre[gi] = di

        w1aT = consts.tile([P, P], f32)
        w1bT = consts.tile([P, P], bf16)
        w2T = consts.tile([P, P], bf16)
        wcT = consts.tile([P, P], bf16)

        with tc.tile_pool(name="psum0", bufs=1, space="PSUM") as psw:
            pw = psw.tile([P, P], f32, tag="pw")
            nc.tensor.matmul(pw[:], lhsT=w1_s[:, 0:P], rhs=ident_f[:],
                             start=True, stop=True)
            nc.vector.tensor_copy(w1aT[:], pw[:])
            pw = psw.tile([P, P], f32, tag="pw")
            nc.tensor.matmul(pw[:], lhsT=w1_s[:, P:2 * P], rhs=ident_f[:],
                             start=True, stop=True)
            nc.vector.tensor_copy(w1bT[:], pw[:])
            pw = psw.tile([P, P], f32, tag="pw")
            nc.tensor.matmul(pw[:], lhsT=w2_s[:], rhs=ident_f[:],
                             start=True, stop=True)
            nc.vector.tensor_copy(w2T[:], pw[:])
            # WcT[i, o] = (W1a @ W_lin)[o, i]
            pw = psw.tile([P, P], f32, tag="pw")
            nc.tensor.matmul(pw[:], lhsT=wlin_s[:], rhs=w1aT[:],
                             start=True, stop=True)
            nc.vector.tensor_copy(wcT[:], pw[:])

        u1 = big.tile([P, ec], bf16)
        stats = consts.tile([P, nchunk, 6], f32)

        def bn_coeffs(g_s, be_s, corr=None):
            se = red.tile([P, nchunk], f32, tag="se")
            nc.vector.tensor_mul(se[:], stats[:, :, 0], stats[:, :, 1])
            so = red.tile([P, nchunk], f32, tag="so")
            nc.vector.tensor_mul(so[:], stats[:, :, 3], stats[:, :, 4])
            qe = red.tile([P, nchunk], f32, tag="qe")
            nc.vector.tensor_mul(qe[:], se[:], stats[:, :, 1])
            nc.vector.tensor_add(qe[:], qe[:], stats[:, :, 2])
            qo = red.tile([P, nchunk], f32, tag="qo")
            nc.vector.tensor_mul(qo[:], so[:], stats[:, :, 4])
            nc.vector.tensor_add(qo[:], qo[:], stats[:, :, 5])
            nc.vector.tensor_add(se[:], se[:], so[:])
            nc.vector.tensor_add(qe[:], qe[:], qo[:])
            sq = red.tile([P, 2], f32, tag="sq")
            nc.vector.tensor_reduce(sq[:, 0:1], se[:],
                                    axis=mybir.AxisListType.X,
                                    op=mybir.AluOpType.add)
            nc.vector.tensor_reduce(sq[:, 1:2], qe[:],
                                    axis=mybir.AxisListType.X,
                                    op=mybir.AluOpType.add)
            cc_in = dram.tile([P, 2], f32, tag="cc_in")
            nc.sync.dma_start(out=cc_in[:], in_=sq[:])
            cc_out = dram.tile([P, 2], f32, tag="cc_out")
            nc.gpsimd.collective_compute(
                "AllReduce", mybir.AluOpType.add, replica_groups=grp_all,
                ins=[cc_in[:].opt()], outs=[cc_out[:].opt()])
            sqg = red.tile([P, 2], f32, tag="sqg")
            nc.sync.dma_start(out=sqg[:], in_=cc_out[:])
            if corr is not None:
                v, vq = corr
                t = red.tile([P, 2], f32, tag="tcorr")
                nc.vector.tensor_scalar_mul(t[:, 0:1], v[:], float(n_pad_tot))
                nc.vector.tensor_scalar_mul(t[:, 1:2], vq[:], float(n_pad_tot))
                nc.vector.tensor_sub(sqg[:], sqg[:], t[:])
            mu = red.tile([P, 1], f32, tag="mu")
            nc.vector.tensor_scalar_mul(mu[:], sqg[:, 0:1], 1.0 / e_total)
            var = red.tile([P, 1], f32, tag="var")
            nc.vector.tensor_scalar_mul(var[:], sqg[:, 1:2], 1.0 / e_total)
            mu2 = red.tile([P, 1], f32, tag="mu2")
            nc.vector.tensor_mul(mu2[:], mu[:], mu[:])
            nc.vector.tensor_sub(var[:], var[:], mu2[:])
            a = red.tile([P, 1], f32, tag="a")
            nc.scalar.activation(a[:], var[:], func=FT.Sqrt, bias=eps_s[:],
                                 scale=1.0)
            nc.vector.reciprocal(a[:], a[:])
            nc.vector.tensor_mul(a[:], a[:], g_s[:])
            c = red.tile([P, 1], f32, tag="c")
            nc.vector.tensor_mul(c[:], mu[:], a[:])
            nc.vector.tensor_sub(c[:], be_s[:], c[:])
            return a, c

        with (
            tc.tile_pool(name="psA", bufs=3, space="PSUM") as psA,
            tc.tile_pool(name="psR", bufs=2, space="PSUM") as psR,
            tc.tile_pool(name="psH", bufs=2, space="PSUM") as psH,
            tc.tile_pool(name="psS", bufs=1, space="PSUM") as psS,
            tc.tile_pool(name="ea", bufs=3) as eap,
            tc.tile_pool(name="gp", bufs=3) as gp,
            tc.tile_pool(name="xw", bufs=3) as xwp,
            tc.tile_pool(name="hw", bufs=3) as hwp,
            tc.tile_pool(name="sw", bufs=3) as swp,
            tc.tile_pool(name="oh", bufs=2) as ohp,
            tc.tile_pool(name="op", bufs=3) as op,
        ):
            # ---- dst gathers --------------------------------------------
            g_tiles = {}
            for gi, (off, L, dst_hi) in enumerate(groups):
                if gi in idx_pre:
                    di = idx_pre[gi]
                else:
                    di = gidx.tile([P, GROUP // 16], i16, tag="di")
                    nc.sync.dma_start(out=di[:, :L // 16],
                                      in_=didx[:, off // 16:(off + L) // 16])
                gdst = gp.tile([P, GROUP], bf16, tag="gdst")
                dst_base = xp[0:hi_rows, :] if dst_hi else xp[hi_rows:npad, :]
                nc.gpsimd.dma_gather(
                    out_ap=gdst[:, :L].rearrange("p (a s) -> p a s", a=1),
                    in_ap=dst_base, idxs_ap=di[:, :L // 16],
                    num_idxs=L, num_idxs_reg=L, elem_size=P,
                    transpose=True, single_packet=False)
                g_tiles[gi] = (gdst, off)

            # ---- pass A --------------------------------------------------
            for k, (off, gi) in enumerate(chunksA):
                gdst, goff = g_tiles[gi]
                rel = off - goff
                S = CHUNK
                ea_t = eap.tile([P, CHUNK], bf16, tag="ea")
                nc.sync.dma_start(out=ea_t[:], in_=eaT[:, off:off + S])
                xw_t = xwp.tile([P, WSPAN], bf16, tag="xw")
                nc.sync.dma_start(out=xw_t[:],
                                  in_=xwins[:, k * WSPAN:(k + 1) * WSPAN])
                sw_t = swp.tile([1, CHUNK], f16, tag="sw")
                nc.sync.dma_start(out=sw_t[:], in_=swin[0:1, off:off + S])

                pr = psR.tile([P, CHUNK], f32, tag="pr")
                nc.tensor.matmul(pr[:], lhsT=ones1[:], rhs=sw_t[:],
                                 start=True, stop=True)
                oha = ohp.tile([P, CHUNK], bf16, tag="oha")
                nc.vector.tensor_scalar(
                    out=oha[:], in0=pr[:], scalar1=iota_f[:], scalar2=None,
                    op0=mybir.AluOpType.is_equal)
                ohb = ohp.tile([P, CHUNK], bf16, tag="ohb")
                nc.vector.tensor_scalar(
                    out=ohb[:], in0=pr[:], scalar1=128.0, scalar2=iota_f[:],
                    op0=mybir.AluOpType.subtract,
                    op1=mybir.AluOpType.is_equal)

                hp = psH.tile([P, WSPAN], f32, tag="hp")
                nc.tensor.matmul(hp[:, 0:P], lhsT=xw_t[:, 0:P], rhs=wcT[:],
                                 start=True, stop=True)
                nc.tensor.matmul(hp[:, P:WSPAN], lhsT=xw_t[:, P:WSPAN],
                                 rhs=wcT[:], start=True, stop=True)
                hw_t = hwp.tile([P, WSPAN], bf16, tag="hw")
                nc.vector.tensor_copy(hw_t[:], hp[:])

                up = psA.tile([P, CHUNK], f32, tag="up")
                nc.tensor.matmul(up[:], lhsT=hw_t[:, 0:P], rhs=oha[:],
                                 start=True, stop=False)
                nc.tensor.matmul(up[:], lhsT=hw_t[:, P:WSPAN], rhs=ohb[:],
                                 start=False, stop=False)
                nc.tensor.matmul(up[:], lhsT=wcT[:], rhs=gdst[:, rel:rel + S],
                                 start=False, stop=False)
                nc.tensor.matmul(up[:], lhsT=w1bT[:], rhs=ea_t[:],
                                 start=False, stop=True)
                nc.scalar.activation(u1[:, off:off + S], up[:],
                                     func=FT.Identity, scale=1.0)
                nc.vector.bn_stats(stats[:, k, :], u1[:, off:off + S])

            a1, c1 = bn_coeffs(g1_s, be1_s)

            # pad columns: u1 == 0 -> u2_pad = W2 @ relu(c1), constant
            rc = red.tile([P, 1], f32, tag="rc")
            nc.scalar.activation(rc[:], c1[:], func=FT.Relu)
            rcb = red.tile([P, 1], bf16, tag="rcb")
            nc.vector.tensor_copy(rcb[:], rc[:])
            vp = psS.tile([P, 1], f32, tag="vp")
            nc.tensor.matmul(vp[:], lhsT=w2T[:], rhs=rcb[:],
                             start=True, stop=True)
            v2 = red.tile([P, 1], f32, tag="v2")
            nc.vector.tensor_copy(v2[:], vp[:])
            v2q = red.tile([P, 1], f32, tag="v2q")
            nc.vector.tensor_mul(v2q[:], v2[:], v2[:])

            # ---- pass B: z1 = relu(a1*u1+c1) in place; stats of W2@z1 ---
            for k in range(nchunk):
                off = k * CHUNK
                sl = u1[:, off:off + CHUNK]
                nc.scalar.activation(sl, sl, func=FT.Relu, scale=a1[:],
                                     bias=c1[:])
                up = psA.tile([P, CHUNK], f32, tag="up")
                nc.tensor.matmul(up[:], lhsT=w2T[:],
                                 rhs=u1[:, off:off + CHUNK],
                                 start=True, stop=True)
                nc.vector.bn_stats(stats[:, k, :], up[:])

            a2, c2 = bn_coeffs(g2_s, be2_s, corr=(v2, v2q))

            # ---- pass C: out = relu(a2*(W2@z1)+c2) ----------------------
            for k in range(nchunk):
                off = k * CHUNK
                up = psA.tile([P, CHUNK], f32, tag="up")
                nc.tensor.matmul(up[:], lhsT=w2T[:],
                                 rhs=u1[:, off:off + CHUNK],
                                 start=True, stop=True)
                ot = op.tile([P, CHUNK], bf16, tag="ot")
                nc.scalar.activation(ot[:], up[:], func=FT.Relu,
                                     scale=a2[:], bias=c2[:])
                nc.sync.dma_start(out=outT[:, off:off + CHUNK], in_=ot[:])

    nc.compile()
    return nc


def _wrap16(a):
    w = np.ascontiguousarray(a.reshape(-1, 16).T)
    return np.tile(w, (8, 1))


def host_prep(x, edge_index, edge_attr, n_cores):
    """Shard by src range; bucket by dst hi/lo; sort by src; pack 512-edge
    chunks with src span < WSPAN; pad to common caps."""
    n = x.shape[0]
    nhi, hi_rows, lo_rows, npad = table_layout(n)
    src_all = edge_index[0].astype(np.int64)
    dst_all = edge_index[1].astype(np.int64)

    zero_lo = SPLIT
    zero_hi = nhi

    per_core = []
    nchunks = np.zeros((n_cores, 2), np.int64)
    for c in range(n_cores):
        ids_c = np.where(src_all // NPC == c)[0]
        s, d = src_all[ids_c], dst_all[ids_c]
        key = (d >= SPLIT).astype(np.int64)
        order = np.lexsort((s, key))
        # chunk packing per bucket: <=512 edges, src span < WSPAN
        chunk_lists = {0: [], 1: []}
        for b in (0, 1):
            idx_b = order[key[order] == b]
            cur = []
            for i in idx_b:
                if cur and (len(cur) == CHUNK or
                            s[i] - s[cur[0]] >= WSPAN):
                    chunk_lists[b].append(cur)
                    cur = []
                cur.append(i)
            if cur:
                chunk_lists[b].append(cur)
        nchunks[c, 0] = len(chunk_lists[0])
        nchunks[c, 1] = len(chunk_lists[1])
        per_core.append((ids_c, s, d, chunk_lists))

    caps = tuple(int(max(1, nchunks[:, b].max())) * CHUNK for b in (0, 1))
    ec = sum(caps)
    offs = {}
    _acc = 0
    for b in BUCKET_ORDER:
        offs[b] = _acc
        _acc += caps[b]

    cores = []
    for c in range(n_cores):
        ids_c, s, d, chunk_lists = per_core[c]
        ne = len(ids_c)
        didx_p = np.empty(ec, np.int64)
        swin_p = np.full(ec, -1.0, np.float16)
        wbase = np.zeros(ec // CHUNK, np.int64)
        ea_cols = np.full(ec, -1, np.int64)  # local edge idx per padded col
        inv = np.empty(ne, np.int64)
        for b in (0, 1):
            # default pads for the whole bucket
            lo = offs[b]
            didx_p[lo:lo + caps[b]] = zero_hi if b == 1 else zero_lo
            for m, members in enumerate(chunk_lists[b]):
                coff = offs[b] + m * CHUNK
                kglob = coff // CHUNK
                sb = s[members]
                base = int(sb.min())
                assert int(sb.max()) - base < WSPAN
                wbase[kglob] = base
                pos = coff + np.arange(len(members))
                db = d[members]
                didx_p[pos] = (db - SPLIT) if b == 1 else db
                swin_p[pos] = (sb - base).astype(np.float16)
                ea_cols[pos] = members
                inv[members] = pos
        cores.append((ids_c, didx_p.astype(np.int16), swin_p, wbase,
                      ea_cols, inv))
    return caps, ec, cores, npad


def make_in_maps(x, edge_index, edge_attr, W_lin, b_lin, W1, g1, be1, W2,
                 g2, be2, n_cores):
    n = x.shape[0]
    nhi, hi_rows, lo_rows, npad = table_layout(n)
    caps, ec, cores, _ = host_prep(x, edge_index, edge_attr, n_cores)

    xbf = np.asarray(x).astype(BF16)
    xp = np.zeros((npad, P), dtype=BF16)
    xp[0:nhi] = xbf[SPLIT:n]
    xp[hi_rows:hi_rows + SPLIT] = xbf[0:SPLIT]
    # feature-major copy for window staging (zero-padded past n)
    xT = np.zeros((P, n + WSPAN), dtype=BF16)
    xT[:, :n] = xbf.T

    f32c = np.ascontiguousarray
    wlin_h = f32c(W_lin.astype(np.float32))
    w1_h = f32c(W1.astype(np.float32))
    w2_h = f32c(W2.astype(np.float32))
    g1_h = f32c(g1.astype(np.float32).reshape(P, 1))
    be1_h = f32c(be1.astype(np.float32).reshape(P, 1))
    g2_h = f32c(g2.astype(np.float32).reshape(P, 1))
    be2_h = f32c(be2.astype(np.float32).reshape(P, 1))

    groups, _ = edge_layout(caps)
    eabf = np.asarray(edge_attr).astype(BF16)

    in_maps = []
    outmaps = []
    for c in range(n_cores):
        ids_c, didx_p, swin_p, wbase, ea_cols, inv = cores[c]
        eaT = np.zeros((P, ec), dtype=BF16)
        real = ea_cols >= 0
        eaT[:, real] = eabf[ids_c[ea_cols[real]]].T
        dw = np.zeros((P, ec // 16), np.int16)
        for off, L, _dh in groups:
            dw[:, off // 16:(off + L) // 16] = _wrap16(didx_p[off:off + L])
        xwins = np.zeros((P, ec // 2), dtype=BF16)
        for kg in range(ec // CHUNK):
            b0 = int(wbase[kg])
            xwins[:, kg * WSPAN:(kg + 1) * WSPAN] = xT[:, b0:b0 + WSPAN]
        in_maps.append({
            "eaT": eaT, "xp": xp, "xwins": xwins,
            "swin": swin_p.reshape(1, ec), "didx": dw,
            "wlin": wlin_h, "w1": w1_h, "w2": w2_h,
            "g1": g1_h, "be1": be1_h, "g2": g2_h, "be2": be2_h,
        })
        outmaps.append((ids_c, inv))
    return caps, ec, in_maps, outmaps


_GRAPH_CACHE = {}


def get_graph(n_cores, caps, n_nodes, e_total):
    key = (n_cores, caps, n_nodes, e_total)
    if key not in _GRAPH_CACHE:
        _GRAPH_CACHE[key] = build_graph(n_cores, caps, n_nodes, e_total)
    return _GRAPH_CACHE[key]


def kernel(x, edge_index, edge_attr, W_lin, b_lin, W1, b1, g1, be1, W2, b2,
           g2, be2):
    x = np.asarray(x)
    edge_index = np.asarray(edge_index)
    edge_attr = np.asarray(edge_attr)
    e = edge_attr.shape[0]
    n = x.shape[0]

    caps, ec, in_maps, outmaps = make_in_maps(
        x, edge_index, edge_attr, np.asarray(W_lin), np.asarray(b_lin),
        np.asarray(W1), np.asarray(g1), np.asarray(be1), np.asarray(W2),
        np.asarray(g2), np.asarray(be2), N_CORES)
    nc = get_graph(N_CORES, caps, n, e)
    res = run_bass_kernel_spmd(nc, in_maps, core_ids=list(range(N_CORES)))
    out = np.empty((e, NIN), dtype=np.float32)
    for c in range(N_CORES):
        oT = np.asarray(res.results[c]["outT"], dtype=np.float32)
        ids_c, inv = outmaps[c]
        out[ids_c] = oT.T[inv]
    return out
